# revision 4
# baseline (speedup 1.0000x reference)
"""Trainium2 Bass kernel v2 for nn_MoEDiscriminator (8 experts, MLP 64->256->256->1).

Data-parallel over 8 NeuronCores (8192 rows/core), transposed layout
([feature_partitions, batch_free]); per 512-col batch tile:

- L1 row-tiled (64x128 PE mode): K=64 contraction, two concurrent matmuls
  (T0: partitions 0-63 / T8: 64-127) per expert -> ~2x L1 throughput.
- L2 full-width 128x128, bf16 weights (fast weight load).
- L3 col-tiled (128x32 mode, positions (0,0) and (0,32) only -- 64/96 fail
  the ISA dst-partition check): experts 0-3 accumulate into pD[0:4],
  experts 4-7 into pD[32:36], two concurrent streams -> ~2x L3.
- Stage-batched per tile, software-pipelined across tiles:
  period i issues [L3(i-3) | L1(i) | L2(i-1)] so PE mode switches are
  3/period and evictions have >=1 period of slack.
- Weights bf16 (error ~0.5% << 2e-2 gate), st f32r, h1/h2 evicted as bf16,
  PSUM accumulation fp32. b1/b2 applied during eviction; b3 host-side.
"""

import sys

sys.path.insert(0, "/opt/trn_rl_repo")
from contextlib import ExitStack

import numpy as np

import concourse.bass as bass
import concourse.tile as tile
from concourse import bacc, mybir
from concourse.bass import ts
from concourse.bass_utils import run_bass_kernel_spmd

P = 128
C = 8            # experts
DS = 64          # input feature dim
H = 256          # hidden width
B = 65536        # full batch
NCORES = 8
NB = B // NCORES  # 8192 rows per core
BT = 512         # batch tile (free dim of matmuls)
MT = 1024        # macro tile: 2 batch tiles per pipeline period
NM = NB // MT    # 8
ST_CHUNKS = (512, 1536, 2048, 4096)   # graduated st chunk widths
PSUM_BUFS = (2, 2)   # psumA/psumB bufs of [128,1024] (2 banks each); pD shares psumA
SPLIT_PARTS = 2                       # L1/L2 interleaved batches (1/2/4)
H1_BUFS = 6
H2_BUFS = 48
L3_DEPTH = 3                          # L3 runs this many periods behind L1
W1_BF16 = True   # st is bf16 too: mixed f32r/bf16 matmuls are rejected by
                 # walrus (NCC_IBIR034), so L1 is pure bf16
ROW_L1 = True                         # row-tiled L1
COL_L3 = True                         # col-tiled L3 (2 groups)

f32 = mybir.dt.float32
f32r = mybir.dt.float32r
bf16 = mybir.dt.bfloat16
AF = mybir.ActivationFunctionType
ALU = mybir.AluOpType

_NC_CACHE = {}


def _build_nc(repeats=1):
    key = (repeats, PSUM_BUFS, H1_BUFS, H2_BUFS, L3_DEPTH, W1_BF16, ROW_L1,
           COL_L3, ST_CHUNKS, SPLIT_PARTS, MT)
    if key in _NC_CACHE:
        return _NC_CACHE[key]
    w1dt = bf16 if W1_BF16 else f32r
    nc = bacc.Bacc("TRN2", target_bir_lowering=False, debug=False,
                   num_devices=NCORES)
    st_d = nc.dram_tensor("st", [P, NB], bf16, kind="ExternalInput").ap()
    w1_d = nc.dram_tensor("w1", [P, C, P], w1dt, kind="ExternalInput").ap()
    w2_d = nc.dram_tensor("w2", [C, 2, 2, P, P], bf16,
                          kind="ExternalInput").ap()
    w3_d = nc.dram_tensor("w3", [P, C, 2, C], bf16, kind="ExternalInput").ap()
    bias_d = nc.dram_tensor("bias", [P, 4 * C], f32, kind="ExternalInput").ap()
    d_d = nc.dram_tensor("d", [C, NB], bf16, kind="ExternalOutput").ap()

    with tile.TileContext(nc) as tc, ExitStack() as ctx:
        const = ctx.enter_context(tc.tile_pool(name="const", bufs=2))
        wh1 = ctx.enter_context(tc.tile_pool(name="wh1", bufs=H1_BUFS))
        wh2 = ctx.enter_context(tc.tile_pool(name="wh2", bufs=H2_BUFS))
        psumA = ctx.enter_context(
            tc.tile_pool(name="psumA", bufs=PSUM_BUFS[0], space="PSUM"))
        psumB = ctx.enter_context(
            tc.tile_pool(name="psumB", bufs=PSUM_BUFS[1], space="PSUM"))

        def body():
            # constants first, smallest-first so compute starts early
            bias_sb = const.tile([P, 4 * C], f32, name="bias_sb")
            nc.sync.dma_start(bias_sb[:], bias_d)
            b1_sb = bias_sb[:, 0:2 * C]
            b2_sb = bias_sb[:, 2 * C:4 * C]
            w1_sb = const.tile([P, C, P], w1dt, name="w1_sb")
            nc.sync.dma_start(w1_sb[:], w1_d)

            st_sb = []
            st_off = []
            off = 0
            for i, cols in enumerate(ST_CHUNKS):
                t_ = const.tile([P, cols], bf16, name=f"st_sb{i}")
                st_sb.append(t_)
                st_off.append(off)
                off += cols
            assert off == NB
            nc.sync.dma_start(st_sb[0][:], st_d[:, 0:ST_CHUNKS[0]])
            w3_sb = const.tile([P, C, 2, C], bf16, name="w3_sb")
            nc.sync.dma_start(w3_sb[:], w3_d)
            nc.sync.dma_start(st_sb[1][:],
                              st_d[:, st_off[1]:st_off[1] + ST_CHUNKS[1]])
            w2_sb = const.tile([P, C, 2, 2, P], bf16, name="w2_sb")
            nc.sync.dma_start(w2_sb[:, 0:4],
                              w2_d[0:4].rearrange("c k j p f -> p c k j f"))
            nc.sync.dma_start(w2_sb[:, 4:8],
                              w2_d[4:8].rearrange("c k j p f -> p c k j f"))
            for i in range(2, len(ST_CHUNKS)):
                nc.sync.dma_start(st_sb[i][:],
                                  st_d[:, st_off[i]:st_off[i] + ST_CHUNKS[i]])

            # experts 0-3 at partitions 0-3, 4-7 at 32-35 (engine partition
            # access must start at a multiple of 32)
            d_sb = const.tile([36, NB], bf16, name="d_sb")

            def st_slice(t, lo, hi):
                col = t * BT
                for i, o in enumerate(st_off):
                    if o <= col < o + ST_CHUNKS[i]:
                        return st_sb[i][lo:hi, col - o:col - o + BT]
                raise AssertionError

            h1s, h2s = {}, {}

            def stage_l1(m, c):
                # pa_k holds both sub-tiles of the macro: bank0=t0, bank1=t1.
                # One FD=1024 eviction per k-half (same bias across banks).
                pa = [psumA.tile([P, MT], f32, tag="pA", name=f"pA{k}")
                      for k in range(2)]
                for t in range(2):
                    tt = 2 * m + t
                    nc.tensor.matmul(pa[0][:, ts(t, BT)], w1_sb[0:64, c, :],
                                     st_slice(tt, 0, 64), start=True, stop=True)
                    nc.tensor.matmul(pa[1][:, ts(t, BT)], w1_sb[64:128, c, :],
                                     st_slice(tt, 64, 128), start=True, stop=True)
                h1 = [wh1.tile([P, MT], bf16, tag="h1", name=f"h1_{k}")
                      for k in range(2)]
                nc.scalar.activation(h1[0][:], pa[0][:], AF.Relu,
                                     bias=b1_sb[:, 2 * c:2 * c + 1])
                nc.vector.tensor_scalar(h1[1][:], pa[1][:],
                                        b1_sb[:, 2 * c + 1:2 * c + 2],
                                        0.0, ALU.add, ALU.max)
                h1s[(m, c)] = h1

            def stage_l2(m, c):
                h1 = h1s.pop((m, c))
                for j in range(2):
                    pb = psumB.tile([P, MT], f32, tag="pB", name=f"pB{j}")
                    # k-major so the W2 stationary is reused across sub-tiles
                    for k in range(2):
                        for t in range(2):
                            nc.tensor.matmul(pb[:, ts(t, BT)],
                                             w2_sb[:, c, k, j, :],
                                             h1[k][:, ts(t, BT)],
                                             start=(k == 0), stop=(k == 1))
                    h2 = wh2.tile([P, MT], bf16, tag="h2", name=f"h2_{j}")
                    if j == 0:
                        nc.scalar.activation(h2[:], pb[:], AF.Relu,
                                             bias=b2_sb[:, 2 * c:2 * c + 1])
                    else:
                        nc.vector.tensor_scalar(h2[:], pb[:],
                                                b2_sb[:, 2 * c + 1:2 * c + 2],
                                                0.0, ALU.add, ALU.max)
                    h2s[(m, c, j)] = h2

            def stage_l3(m):
                # pd lives in psumB's rotation: psumA keeps both bufs free
                # for the L1a burst that follows the L3 phase
                pd = psumB.tile([P, MT], f32, tag="pB", name="pD")
                grabbed = {}
                for e in range(4):
                    for k in range(2):
                        for g in range(2):
                            c = 4 * g + e
                            if (c, k) not in grabbed:
                                grabbed[(c, k)] = h2s.pop((m, c, k))
                            h2 = grabbed[(c, k)]
                            for t in range(2):
                                nc.tensor.matmul(
                                    pd[32 * g:32 * g + 4, ts(t, BT)],
                                    w3_sb[:, c, k, 4 * g:4 * g + 4],
                                    h2[:, ts(t, BT)],
                                    start=(e == 0 and k == 0),
                                    stop=(e == 3 and k == 1),
                                    tile_position=(0, 32 * g))
                # split across engines so neither queue delays h1 evictions
                nc.scalar.copy(d_sb[0:4, ts(m, MT)], pd[0:4, :])
                nc.vector.tensor_copy(d_sb[32:36, ts(m, MT)], pd[32:36, :])

            for i in range(NM + L3_DEPTH):
                if i >= L3_DEPTH:
                    stage_l3(i - L3_DEPTH)
                # [L3|L1a|L2a|L1b|L2b]: L1's PSUM evictions drain during
                # the adjacent L2 phase instead of stalling the L1 burst
                w = C // SPLIT_PARTS
                for part in range(SPLIT_PARTS):
                    if i < NM:
                        for c in range(w * part, w * part + w):
                            stage_l1(i, c)
                    if 1 <= i <= NM:
                        for c in range(w * part, w * part + w):
                            stage_l2(i - 1, c)
            if COL_L3:
                nc.sync.dma_start(d_d[0:4], d_sb[0:4, :])
                nc.sync.dma_start(d_d[4:8], d_sb[32:36, :])
            else:
                nc.sync.dma_start(d_d, d_sb[0:8, :])

        for _rep in range(repeats):
            body()

    nc.compile()
    _NC_CACHE[key] = nc
    return nc


def _prep_weights(W1, b1, W2, b2, W3):
    import ml_dtypes
    bfd = ml_dtypes.bfloat16
    w1dt = bfd if W1_BF16 else np.float32
    w1p = np.zeros((P, C, P), w1dt)
    for c in range(C):
        w1p[0:64, c, :] = W1[c][:, 0:128].astype(w1dt)
        w1p[64:128, c, :] = W1[c][:, 128:256].astype(w1dt)
    w2p = np.ascontiguousarray(
        W2.reshape(C, 2, P, 2, P).transpose(0, 1, 3, 2, 4)).astype(bfd)
    w3p = np.zeros((P, C, 2, C), bfd)
    for c in range(C):
        for k in range(2):
            w3p[:, c, k, c] = W3[c, k * P:(k + 1) * P, 0].astype(bfd)
    b1h = np.ascontiguousarray(b1.reshape(C * 2, P).T)  # [128, C*2]
    b2h = np.ascontiguousarray(b2.reshape(C * 2, P).T)
    biasp = np.ascontiguousarray(np.concatenate([b1h, b2h], axis=1),
                                 dtype=np.float32)
    return w1p, w2p, w3p, biasp


def _make_in_maps(st, W1, b1, W2, b2, W3):
    w1p, w2p, w3p, biasp = _prep_weights(W1, b1, W2, b2, W3)
    in_maps = []
    for core in range(NCORES):
        shard = st[core * NB:(core + 1) * NB]            # [8192, 64]
        import ml_dtypes
        stT = np.ascontiguousarray(
            np.concatenate([shard.T, shard.T], axis=0).astype(
                ml_dtypes.bfloat16))                      # [128, 8192]
        in_maps.append({"st": stT, "w1": w1p, "w2": w2p, "w3": w3p,
                        "bias": biasp})
    return in_maps


class _SpmdExec:
    """Reusable jitted shard_map executor for a compiled Bass module."""

    def __init__(self, nc, n_cores):
        import jax
        from jax.sharding import Mesh, PartitionSpec
        from jax.experimental.shard_map import shard_map
        from concourse.bass2jax import (_bass_exec_p, partition_id_tensor,
                                        install_neuronx_cc_hook)

        install_neuronx_cc_hook()
        self.n_cores = n_cores
        in_names, out_names, out_avals = [], [], []
        pname = nc.partition_id_tensor.name if nc.partition_id_tensor else None
        for alloc in nc.m.functions[0].allocations:
            if not isinstance(alloc, mybir.MemoryLocationSet):
                continue
            name = alloc.memorylocations[0].name
            if alloc.kind == "ExternalInput":
                if name != pname:
                    in_names.append(name)
            elif alloc.kind == "ExternalOutput":
                out_names.append(name)
                out_avals.append(jax.core.ShapedArray(
                    tuple(alloc.tensor_shape), mybir.dt.np(alloc.dtype)))
        self.in_names, self.out_names, self.out_avals = \
            in_names, out_names, out_avals
        all_in = in_names + out_names + ([pname] if pname else [])

        def _bdy(*args):
            ops = list(args)
            if pname is not None:
                ops.append(partition_id_tensor())
            return tuple(_bass_exec_p.bind(
                *ops, out_avals=tuple(out_avals), in_names=tuple(all_in),
                out_names=tuple(out_names), lowering_input_output_aliases=(),
                sim_require_finite=True, sim_require_nnan=True, nc=nc))

        mesh = Mesh(np.asarray(jax.devices()[:n_cores]), ("core",))
        nio = len(in_names) + len(out_names)
        self.sharded = jax.jit(
            shard_map(_bdy, mesh=mesh,
                      in_specs=(PartitionSpec("core"),) * nio,
                      out_specs=(PartitionSpec("core"),) * len(out_names),
                      check_rep=False),
            keep_unused=True)

    def run(self, in_maps):
        args = [np.concatenate([np.asarray(m[n]) for m in in_maps], axis=0)
                for n in self.in_names]
        args += [np.zeros((self.n_cores * a.shape[0], *a.shape[1:]), a.dtype)
                 for a in self.out_avals]
        outs = self.sharded(*args)
        return [{n: np.asarray(outs[i]).reshape(
                    self.n_cores, *self.out_avals[i].shape)[c]
                 for i, n in enumerate(self.out_names)}
                for c in range(self.n_cores)]


_EXEC_CACHE = {}


def _run_spmd(nc, in_maps, first_call):
    if not first_call:
        ex = _EXEC_CACHE.get(id(nc))
        if ex is None:
            ex = _EXEC_CACHE[id(nc)] = _SpmdExec(nc, NCORES)
        return ex.run(in_maps)
    import os
    try:
        return run_bass_kernel_spmd(
            nc, in_maps, core_ids=list(range(NCORES))).results
    except ModuleNotFoundError:
        os.environ["BASS_NEVER_TRACE"] = "1"
        return run_bass_kernel_spmd(
            nc, in_maps, core_ids=list(range(NCORES))).results


_CALLED = False


def kernel(st, W1, b1, W2, b2, W3, b3):
    global _CALLED
    st = np.ascontiguousarray(np.asarray(st, np.float32))
    in_maps = _make_in_maps(
        st,
        np.asarray(W1, np.float32), np.asarray(b1, np.float32),
        np.asarray(W2, np.float32), np.asarray(b2, np.float32),
        np.asarray(W3, np.float32))
    nc = _build_nc(1)
    results = _run_spmd(nc, in_maps, first_call=not _CALLED)
    _CALLED = True

    b3v = np.asarray(b3, np.float32).reshape(1, C)
    out = np.empty((B, C, 1), np.float32)
    for core in range(NCORES):
        d = np.asarray(results[core]["d"], np.float32)    # [8, 8192]
        out[core * NB:(core + 1) * NB, :, 0] = d.T + b3v
    return out
